# revision 10
# baseline (speedup 1.0000x reference)
"""BQuantConv1d Trainium2 kernel.

Math: the reference's per-token LUT + gather is algebraically a matmul:
  out[n, f] = sum_i x[n, i] * W[i, f] + bias[f]
  W[8g+j, f] = sum_b scale[b, f] * (2*bit_{7-j}(binary[b, g, f]) - 1)

Sharding: 2 token-groups x 4 f-groups over 8 cores, no collectives
(host slices inputs / concatenates outputs; layout-only host work).
Contraction order is permuted to i' = j*128 + g (host permutes xT rows to
match) so each decoded weight chunk j lands on contiguous partitions.

Per core:
  - decode W'(1024, 256) from int16 codes with a sign-bit trick:
    W element = +-scale[b, f] exactly, built by XORing the fp16 scale's
    sign bit (scales arrive sign-pre-flipped) with the masked quant bit
    (c << (8+j)) & 0x8000, as int32 SWAR on DVE (bitvec ops are DVE-only
    and 32-bit-only on walrus); the 8-way b-reduction is an fp16 add
    tree with the first level (h1) on DVE and the h2/w levels offloaded
    to the otherwise-idle GPSIMD engine.  Chunks 0/1 run TS/xor in
    b-halves gated on half-sized cd/sc DMAs (starts the decode ~0.8us
    earlier); each later chunk's TS is emitted between xor_j and h1_j so
    it fills the write-ack window and the greedy scheduler doesn't park
    the next 1.1us xor in front of h1_j; chunk 7's tree stays on DVE
    (GPSIMD latency would gate the tail) split by f-half so the fb=0
    matmuls start while fb=1 is still reducing;
  - outT[f_shard, n_shard] = W'.T @ xT on the PE in fp16, accumulating
    the 8 contraction chunks across 8 concurrent PSUM banks (f32);
    each bank is seeded with the bias via a K=1 bias x ones matmul --
    the seeds run in the pipeline head while the PE is otherwise idle
    and double as its p-state warmup;
  - PSUM pairs evacuated as fp16 (copies alternating DVE/ACT) into
    double-wide tiles; the out DRAM layout is token-major per f-block,
    so each ch-pair ships as ONE partition-major DMA (4 output DMAs
    instead of 8 -- the issue+HWDGE pipe, not bytes, dominates the
    tail), on the SP queue except one mid-stream pair on GPSIMD SWDGE.

Cost-model notes (TimelineSim, the graded metric): DVE is the
bottleneck engine (~19us busy: TS 8B/cyc, TT-xor 4B/cyc, fp16 adds
4B/cyc); total = DVE-end + ~8us of structural head/tail (2us DMA issue
pipe + 0.9us DMA-completion sem props on the head; 16-matmul PE drain +
evac chains + serialized output DMA transfers + 0.9us sem on the tail).
"""

import numpy as np

try:
    import concourse.bass as bass  # noqa: F401
except ImportError:
    import sys

    sys.path.insert(0, "/opt/trn_rl_repo")
    import concourse.bass as bass  # noqa: F401

import concourse.bacc as bacc
import concourse.mybir as mybir
import concourse.tile as tile

B, T, NX, NF = 2, 2048, 1024, 1024
N_TOK = B * T
BITS = 8
G = NX // 8  # 128 code groups
PT, PF = 2, 4  # token-parallel x feature-parallel
TOK = N_TOK // PT  # tokens per core
NFS = NF // PF  # output features per core
P = 128
MM_N = 512  # moving free dim per matmul

AX = mybir.AxisListType
OP = mybir.AluOpType
F32 = mybir.dt.float32
BF16 = mybir.dt.float16  # compute dtype (fp16: same SWAR, more mantissa)
I16 = mybir.dt.int16
I32 = mybir.dt.int32
ACT_F = mybir.ActivationFunctionType
BF16NP = np.float16

MSK = -2147450880  # 0x80008000 as int32


def build_graph(nc, tok=TOK, nfs=NFS):
    nfb = nfs // P  # f blocks of 128 (2)
    nch = tok // MM_N  # moving chunks (4)
    xt_d = nc.dram_tensor("xt", (8, P, tok), BF16, kind="ExternalInput")
    cd_d = nc.dram_tensor("codes", (P, 8 * nfs), I16, kind="ExternalInput")
    sc_d = nc.dram_tensor("scales", (P, 8 * nfs), BF16, kind="ExternalInput")
    bi_d = nc.dram_tensor("biasc", (P, nfb), F32, kind="ExternalInput")
    out_d = nc.dram_tensor("out", (nfb, P, tok), BF16, kind="ExternalOutput")

    with tile.TileContext(nc) as tc:
        with (
            tc.tile_pool(name="xp", bufs=8) as xp,
            tc.tile_pool(name="cp", bufs=8) as cp,
            tc.tile_pool(name="wp", bufs=8) as wp,
            tc.tile_pool(name="qp", bufs=6) as qp,
            tc.tile_pool(name="cst", bufs=1) as cst,
            tc.tile_pool(name="op", bufs=8) as op_,
            tc.tile_pool(name="pp", bufs=8, space="PSUM") as pp,
        ):
            # --- loads; codes/scales first, in (b-)quarters: the decode's
            # first TS runs on the first cd quarter ~0.7us earlier than a
            # half-DMA gate would allow; bias column first (tiny) ---
            H = 4 * nfs  # b-half
            Q = 2 * nfs  # b-quarter
            cd = cp.tile([P, 8 * nfs], I16, tag="cd")
            sc_bc = cst.tile([P, 8 * nfs], BF16, tag="sc_bc")
            biasc = cst.tile([P, nfb], F32, tag="biasc")
            nc.sync.dma_start(biasc[:], bi_d[:])
            for q in range(4):
                lo, hi = q * Q, (q + 1) * Q
                nc.sync.dma_start(cd[:, lo:hi], cd_d[:, lo:hi])
                nc.sync.dma_start(sc_bc[:, lo:hi], sc_d[:, lo:hi])
            xts = []
            for j in range(8):
                xt = xp.tile([P, tok], BF16, tag="xt")
                nc.sync.dma_start(xt[:], xt_d[j])
                xts.append(xt)

            # PSUM banks; j=0's matmuls run with start=True (no bias seeds:
            # the bias rides the evacuation ops for free)
            pss = {}
            for fb in range(nfb):
                for ch in range(nch):
                    ps = pp.tile([P, MM_N], F32, tag="ps", name=f"ps{fb}_{ch}")
                    pss[(fb, ch)] = ps

            # --- decode W chunks ---
            # Sign-bit trick: masked quant bit (inverted) XORed onto the
            # fp16 scale's sign gives +-scale exactly.  Bitvec ops are
            # DVE-only and 32-bit-only on walrus, so they run as int32 SWAR
            # over int16-lane pairs: a left shift by 8+j sources each
            # lane's bit 15 from within the same lane, and the 0x80008000
            # mask keeps only the two sign bits.  The bit inversion is
            # folded into a one-time sign-flip of the scale tile:
            #   ((c << (8+j)) & M) ^ (sc ^ M)  ==  ((~c << (8+j)) & M) ^ sc
            def emit_ts(j):
                sg = qp.tile([P, 8 * nfs], I16, tag="sg", name=f"sg{j}")
                nc.vector.tensor_scalar(
                    sg[:].bitcast(I32), cd[:].bitcast(I32), 8 + j, MSK,
                    OP.logical_shift_left, OP.bitwise_and,
                )
                return sg

            def emit_xor(j, sg):
                wsg = qp.tile([P, 8 * nfs], I16, tag="wsg", name=f"wsg{j}")
                nc.vector.tensor_tensor(
                    wsg[:].bitcast(I32), sg[:].bitcast(I32),
                    sc_bc[:].bitcast(I32), OP.bitwise_xor,
                )
                return wsg

            def emit_h1(j, wsg):
                wv = wsg[:].bitcast(BF16)
                h1 = qp.tile([P, 4 * nfs], BF16, tag="h1", name=f"h1_{j}")
                nc.vector.tensor_tensor(
                    h1[:], wv[:, : 4 * nfs], wv[:, 4 * nfs :], OP.add
                )
                return h1

            def tree_tail(j, h1, teng):
                h2 = qp.tile([P, 2 * nfs], BF16, tag="h2", name=f"h2_{j}")
                teng.tensor_tensor(
                    h2[:], h1[:, : 2 * nfs], h1[:, 2 * nfs :], OP.add
                )
                w = wp.tile([P, nfs], BF16, tag="w", name=f"w{j}")
                teng.tensor_tensor(w[:], h2[:, :nfs], h2[:, nfs:], OP.add)
                return w

            def emit_h1_pool(j, wsg):
                wv = wsg[:].bitcast(BF16)
                h1 = qp.tile([P, 4 * nfs], BF16, tag="h1", name=f"h1_{j}")
                nc.gpsimd.tensor_tensor(
                    h1[:], wv[:, : 4 * nfs], wv[:, 4 * nfs :], OP.add
                )
                return h1

            # b-reduction fp16 add tree: h1 on DVE (chunk 3's on the
            # otherwise-slack GPSIMD, shaving ~0.65us off the DVE stream);
            # h2/w on GPSIMD for chunks 0-6.  The next chunk's TS is
            # emitted BETWEEN xor_j and h1_j: it is always ready, so it
            # fills the write-ack window after xor_j and the scheduler then
            # runs h1_j instead of parking the next 1.1us xor in front of
            # it.  Chunk 7's whole tree stays on DVE (GPSIMD latency would
            # gate the tail) split by f-half at every level so the fb=0
            # matmuls start while fb=1 is still reducing.
            ws = {}
            # chunk 0 in b-quarters, chunk 1 in b-halves: each slice starts
            # on its slice-DMA (fills the DVE while later cd/sc slices are
            # still in flight)
            sgs, wsgs = {}, {}
            for j, nsl in ((0, 4), (1, 2)):
                sg = qp.tile([P, 8 * nfs], I16, tag="sg", name=f"sg{j}")
                wsg = qp.tile([P, 8 * nfs], I16, tag="wsg", name=f"wsg{j}")
                SL = (8 * nfs) // nsl
                for sl in range(nsl):
                    lo, hi = sl * SL, (sl + 1) * SL
                    nc.vector.tensor_scalar(
                        sg[:, lo:hi].bitcast(I32), cd[:, lo:hi].bitcast(I32),
                        8 + j, MSK, OP.logical_shift_left, OP.bitwise_and,
                    )
                    nc.vector.tensor_tensor(
                        wsg[:, lo:hi].bitcast(I32),
                        sg[:, lo:hi].bitcast(I32),
                        sc_bc[:, lo:hi].bitcast(I32), OP.bitwise_xor,
                    )
                sgs[j], wsgs[j] = sg, wsg
            POOL_H1 = (3,)
            for j in range(1, 8):
                if j > 1:
                    sgs[j] = emit_ts(j)
                if j - 1 in POOL_H1:
                    h1 = emit_h1_pool(j - 1, wsgs[j - 1])
                else:
                    h1 = emit_h1(j - 1, wsgs[j - 1])
                ws[j - 1] = tree_tail(j - 1, h1, nc.gpsimd)
                if j > 1:
                    wsgs[j] = emit_xor(j, sgs[j])
            # chunk 7: all three tree levels split by f-half on DVE so
            # W7[fb0] lands ~0.6us after xor7 and the PE tail starts early
            wv7 = wsgs[7][:].bitcast(BF16).rearrange("p (b f) -> p b f", b=8)
            w7 = wp.tile([P, nfs], BF16, tag="w", name="w7")
            for half in range(2):
                f0, f1 = half * P, (half + 1) * P
                h1h = qp.tile([P, 4, P], BF16, tag="h1h", name=f"h1h{half}")
                nc.vector.tensor_tensor(
                    h1h[:], wv7[:, :4, f0:f1], wv7[:, 4:, f0:f1], OP.add
                )
                h2h = qp.tile([P, 2, P], BF16, tag="h2h", name=f"h2h{half}")
                nc.vector.tensor_tensor(
                    h2h[:], h1h[:, :2], h1h[:, 2:], OP.add
                )
                nc.vector.tensor_tensor(
                    w7[:, f0:f1], h2h[:, 0], h2h[:, 1], OP.add
                )
            ws[7] = w7

            # --- matmul: outT[f, n] = bias + sum_j W_j.T @ xT_j ---
            # j outermost: each W chunk feeds the PE as soon as it is
            # decoded, all nfb*nch PSUM banks accumulate concurrently.
            # The last chunk is issued bank-by-bank so evacuation and
            # output DMA overlap the remaining j=7 matmuls.
            for j in range(7):
                for fb in range(nfb):
                    for ch in range(nch):
                        nc.tensor.matmul(
                            pss[(fb, ch)][:],
                            ws[j][:, fb * P : (fb + 1) * P],
                            xts[j][:, ch * MM_N : (ch + 1) * MM_N],
                            start=(j == 0),
                            stop=False,
                        )
            # evacuation alternates DVE/ACT (GPSIMD cannot read PSUM) and
            # folds the bias in for free: DVE as tensor_scalar add with the
            # per-partition fp32 bias column, ACT as Identity with the AP
            # bias operand.  Banks pair up into double-wide tiles; each
            # pair ships as ONE DMA, alternating the SP (HWDGE) and gpsimd
            # (SWDGE) queues so the two issue pipes overlap in the tail.
            k = 0
            for fb in range(nfb):
                bcol = biasc[:, fb : fb + 1]
                for cp2 in range(nch // 2):
                    obw = op_.tile([P, 2 * MM_N], BF16, tag="obw",
                                   name=f"obw{fb}_{cp2}")
                    for half in range(2):
                        ch = 2 * cp2 + half
                        nc.tensor.matmul(
                            pss[(fb, ch)][:],
                            ws[7][:, fb * P : (fb + 1) * P],
                            xts[7][:, ch * MM_N : (ch + 1) * MM_N],
                            start=False,
                            stop=True,
                        )
                        dst = obw[:, half * MM_N : (half + 1) * MM_N]
                        if k % 2 == 0:
                            nc.vector.tensor_scalar(
                                dst, pss[(fb, ch)][:], bcol, None, OP.add
                            )
                        else:
                            nc.scalar.activation(
                                dst, pss[(fb, ch)][:], ACT_F.Identity,
                                bias=bcol,
                            )
                        k += 1
                    deng = nc.gpsimd if cp2 % 2 == 1 else nc.sync
                    deng.dma_start(
                        out_d[fb][:, 2 * cp2 * MM_N : (2 * cp2 + 2) * MM_N],
                        obw[:],
                    )
    nc.compile()
    return nc


_I_PERM = 8 * (np.arange(NX) % G) + np.arange(NX) // G  # i' -> i


def host_prep(x, binary, scale, bias):
    """Layout-only sharding (plus x's fp16 compute-precision cast).
    Returns in_maps for cores 0..7 (pt = c//PF, pf = c%PF)."""
    x2 = np.ascontiguousarray(x.reshape(N_TOK, NX).T)[_I_PERM]  # (NX, N)
    x2 = x2.astype(BF16NP)  # compute dtype
    binary16 = binary.astype(np.int16)  # lossless: codes are 0..255
    in_maps = []
    for c in range(8):
        pt, pf = c // PF, c % PF
        f0 = pf * NFS
        xs = np.ascontiguousarray(x2[:, pt * TOK : (pt + 1) * TOK]).reshape(
            8, P, TOK
        )
        cs = np.ascontiguousarray(
            binary16[:, :, f0 : f0 + NFS].transpose(1, 0, 2)
        ).reshape(P, 8 * NFS)
        ss = np.ascontiguousarray(
            np.broadcast_to(
                (-scale[:, f0 : f0 + NFS].astype(BF16NP)).reshape(1, 8 * NFS),
                (P, 8 * NFS),
            )
        )
        bs = np.ascontiguousarray(
            bias[f0 : f0 + NFS].astype(np.float32).reshape(NFS // P, P).T
        )
        in_maps.append({"xt": xs, "codes": cs, "scales": ss, "biasc": bs})
    return in_maps


def host_assemble(results):
    """results[c]["out"]: (NFB, 128, TOK) -> full (B, T, NF)."""
    outT = np.empty((NF, N_TOK), dtype=np.float32)
    for c in range(8):
        pt, pf = c // PF, c % PF
        o = np.asarray(results[c]["out"], dtype=np.float32).reshape(NFS, TOK)
        outT[pf * NFS : (pf + 1) * NFS, pt * TOK : (pt + 1) * TOK] = o
    return np.ascontiguousarray(outT.T).reshape(B, T, NF)


_NC_CACHE = {}


def _get_nc():
    if "nc" not in _NC_CACHE:
        nc = bacc.Bacc(None, target_bir_lowering=False)
        build_graph(nc)
        _NC_CACHE["nc"] = nc
    return _NC_CACHE["nc"]


def kernel(**inputs):
    from concourse.bass_utils import run_bass_kernel_spmd

    inputs = {k: np.asarray(v) for k, v in inputs.items()}
    in_maps = host_prep(
        inputs["x"], inputs["binary"], inputs["scale"], inputs["bias"]
    )
    res = run_bass_kernel_spmd(_get_nc(), in_maps, core_ids=list(range(8)))
    return host_assemble(res.results)



# revision 11
# speedup vs baseline: 1.0908x; 1.0908x over previous
"""BQuantConv1d Trainium2 kernel.

Math: the reference's per-token LUT + gather is algebraically a matmul:
  out[n, f] = sum_i x[n, i] * W[i, f] + bias[f]
  W[8g+j, f] = sum_b scale[b, f] * (2*bit_{7-j}(binary[b, g, f]) - 1)

Sharding: 2 token-groups x 4 f-groups over 8 cores, no collectives
(host slices inputs / concatenates outputs; layout-only host work).
Contraction order is permuted to i' = j*128 + g (host permutes xT rows to
match) so each decoded weight chunk j lands on contiguous partitions.

Per core:
  - decode W'(1024, 256) from int16 codes with a sign-bit trick:
    W element = +-scale[b, f] exactly, built by XORing the fp16 scale's
    sign bit (scales arrive sign-pre-flipped) with the masked quant bit
    (c << (8+j)) & 0x8000, as int32 SWAR on DVE (bitvec ops are DVE-only
    and 32-bit-only on walrus); the 8-way b-reduction is an fp16 add
    tree with the first level (h1) on DVE and the h2/w levels offloaded
    to the otherwise-idle GPSIMD engine.  Chunks 0/1 run TS/xor in
    b-halves gated on half-sized cd/sc DMAs; chunk 7's whole tree runs
    on DVE split by f-half at every level so the fb=0 matmuls start
    while fb=1 is still reducing.
  - outT[f_shard, n_shard] = W'.T @ xT on the PE in fp16, accumulating
    the 8 contraction chunks across 8 concurrent PSUM banks (f32).
  - The PE p-state ramp (cost model: ~7us of continuous execution
    before full clock) is paid with low-priority zero-matmul fillers
    that keep the PE busy from ~1us instead of bias-seed matmuls; the
    filler bank's real accumulation chain is dep-deferred behind them
    and doubles as tail work.  The bias rides the PSUM evacuation for
    free (DVE tensor_scalar add with a per-partition fp32 bias column /
    ACT Identity activation with an AP bias operand).
  - PSUM pairs evacuated as fp16 (copies alternating DVE/ACT) into
    double-wide tiles; each ch-pair ships as ONE partition-major DMA,
    fb0 pairs on the gpsimd SWDGE queue, fb1 pairs on SP/HWDGE, so the
    two issue pipes overlap in the tail.
"""

import numpy as np

try:
    import concourse.bass as bass  # noqa: F401
except ImportError:
    import sys

    sys.path.insert(0, "/opt/trn_rl_repo")
    import concourse.bass as bass  # noqa: F401

import concourse.bacc as bacc
import concourse.mybir as mybir
import concourse.tile as tile

B, T, NX, NF = 2, 2048, 1024, 1024
N_TOK = B * T
BITS = 8
G = NX // 8  # 128 code groups
PT, PF = 2, 4  # token-parallel x feature-parallel
TOK = N_TOK // PT  # tokens per core
NFS = NF // PF  # output features per core
P = 128
MM_N = 512  # moving free dim per matmul
N_FILL = 44  # PE warm-up/pacing zero-matmuls (tuned against TimelineSim)

AX = mybir.AxisListType
OP = mybir.AluOpType
F32 = mybir.dt.float32
BF16 = mybir.dt.float16  # compute dtype (fp16: same SWAR, more mantissa)
I16 = mybir.dt.int16
I32 = mybir.dt.int32
ACT_F = mybir.ActivationFunctionType
BF16NP = np.float16

MSK = -2147450880  # 0x80008000 as int32


def build_graph(nc, tok=TOK, nfs=NFS):
    nfb = nfs // P  # f blocks of 128 (2)
    nch = tok // MM_N  # moving chunks (4)
    xt_d = nc.dram_tensor("xt", (8, P, tok), BF16, kind="ExternalInput")
    cd_d = nc.dram_tensor("codes", (P, 8 * nfs), I16, kind="ExternalInput")
    sc_d = nc.dram_tensor("scales", (P, 8 * nfs), BF16, kind="ExternalInput")
    bi_d = nc.dram_tensor("biasc", (P, nfb), F32, kind="ExternalInput")
    out_d = nc.dram_tensor("out", (nfb, P, tok), BF16, kind="ExternalOutput")
    FILLB = (nfb - 1, nch - 1)  # bank whose real chain hides behind fillers

    with tile.TileContext(nc) as tc:
        with (
            tc.tile_pool(name="xp", bufs=8) as xp,
            tc.tile_pool(name="cp", bufs=8) as cp,
            tc.tile_pool(name="wp", bufs=8) as wp,
            tc.tile_pool(name="qp", bufs=6) as qp,
            tc.tile_pool(name="cst", bufs=1) as cst,
            tc.tile_pool(name="op", bufs=8) as op_,
            tc.tile_pool(name="pp", bufs=8, space="PSUM") as pp,
        ):
            # --- loads; codes/scales first, halves interleaved: chunk 0/1's
            # decode runs in b-halves gated on each half-DMA ---
            H = 4 * nfs
            cd = cp.tile([P, 8 * nfs], I16, tag="cd")
            sc_bc = cst.tile([P, 8 * nfs], BF16, tag="sc_bc")
            nc.sync.dma_start(cd[:, :H], cd_d[:, :H])
            nc.sync.dma_start(sc_bc[:, :H], sc_d[:, :H])
            nc.sync.dma_start(cd[:, H:], cd_d[:, H:])
            nc.sync.dma_start(sc_bc[:, H:], sc_d[:, H:])
            biasc = cst.tile([P, nfb], F32, tag="biasc")
            nc.sync.dma_start(biasc[:], bi_d[:])
            zero_m = cst.tile([P, MM_N], BF16, tag="zero_m")
            nc.gpsimd.memset(zero_m[:], 0.0)
            xts = []
            for j in range(8):
                xt = xp.tile([P, tok], BF16, tag="xt")
                nc.sync.dma_start(xt[:], xt_d[j])
                xts.append(xt)

            # PSUM banks.  FILLB is reset by the first zero-filler; every
            # other bank starts accumulation at its j=0 matmul.
            pss = {}
            for fb in range(nfb):
                for ch in range(nch):
                    ps = pp.tile([P, MM_N], F32, tag="ps", name=f"ps{fb}_{ch}")
                    pss[(fb, ch)] = ps

            # --- PE warm-up / pacing fillers: zero-weight matmuls into
            # FILLB.  Low priority: the scheduler runs them only when no
            # real matmul is ready, keeping the PE's p-state ramp warm
            # from ~1us.  FILLB's real j-chain is dep-ordered behind them
            # and becomes tail work the PE would otherwise idle through.
            with tc.high_priority(offset=-(10**6)):
                nc.tensor.matmul(
                    pss[FILLB][:], zero_m[:, :P], zero_m[:],
                    start=True, stop=False,
                )
                for i in range(N_FILL - 1):
                    nc.tensor.matmul(
                        pss[FILLB][:], zero_m[:, :P], zero_m[:],
                        start=False, stop=False,
                    )

            # --- decode W chunks ---
            # Sign-bit trick: masked quant bit (inverted) XORed onto the
            # fp16 scale's sign gives +-scale exactly.  Bitvec ops are
            # DVE-only and 32-bit-only on walrus, so they run as int32 SWAR
            # over int16-lane pairs: a left shift by 8+j sources each
            # lane's bit 15 from within the same lane, and the 0x80008000
            # mask keeps only the two sign bits.  The bit inversion is
            # folded into a one-time sign-flip of the scale tile:
            #   ((c << (8+j)) & M) ^ (sc ^ M)  ==  ((~c << (8+j)) & M) ^ sc
            def emit_ts(j):
                sg = qp.tile([P, 8 * nfs], I16, tag="sg", name=f"sg{j}")
                nc.vector.tensor_scalar(
                    sg[:].bitcast(I32), cd[:].bitcast(I32), 8 + j, MSK,
                    OP.logical_shift_left, OP.bitwise_and,
                )
                return sg

            def emit_xor(j, sg):
                wsg = qp.tile([P, 8 * nfs], I16, tag="wsg", name=f"wsg{j}")
                nc.vector.tensor_tensor(
                    wsg[:].bitcast(I32), sg[:].bitcast(I32),
                    sc_bc[:].bitcast(I32), OP.bitwise_xor,
                )
                return wsg

            def emit_h1(j, wsg):
                wv = wsg[:].bitcast(BF16)
                h1 = qp.tile([P, 4 * nfs], BF16, tag="h1", name=f"h1_{j}")
                nc.vector.tensor_tensor(
                    h1[:], wv[:, : 4 * nfs], wv[:, 4 * nfs :], OP.add
                )
                return h1

            def tree_tail(j, h1, teng):
                h2 = qp.tile([P, 2 * nfs], BF16, tag="h2", name=f"h2_{j}")
                teng.tensor_tensor(
                    h2[:], h1[:, : 2 * nfs], h1[:, 2 * nfs :], OP.add
                )
                w = wp.tile([P, nfs], BF16, tag="w", name=f"w{j}")
                teng.tensor_tensor(w[:], h2[:, :nfs], h2[:, nfs:], OP.add)
                return w

            # b-reduction fp16 add tree: h1 on DVE; h2/w on GPSIMD for
            # chunks 0-6.  The next chunk's TS is emitted BETWEEN xor_j and
            # h1_j: it is always ready, so it fills the write-ack window
            # after xor_j and the scheduler then runs h1_j instead of
            # parking the next 1.1us xor in front of it.
            ws = {}
            # chunks 0/1 in b-halves so each half starts on its half-DMA
            # (fills the DVE while the second cd/sc halves are in flight)
            sgs, wsgs = {}, {}
            for j in (0, 1):
                sg = qp.tile([P, 8 * nfs], I16, tag="sg", name=f"sg{j}")
                wsg = qp.tile([P, 8 * nfs], I16, tag="wsg", name=f"wsg{j}")
                for half in range(2):
                    lo, hi = half * H, (half + 1) * H
                    nc.vector.tensor_scalar(
                        sg[:, lo:hi].bitcast(I32), cd[:, lo:hi].bitcast(I32),
                        8 + j, MSK, OP.logical_shift_left, OP.bitwise_and,
                    )
                    nc.vector.tensor_tensor(
                        wsg[:, lo:hi].bitcast(I32),
                        sg[:, lo:hi].bitcast(I32),
                        sc_bc[:, lo:hi].bitcast(I32), OP.bitwise_xor,
                    )
                sgs[j], wsgs[j] = sg, wsg
            for j in range(1, 8):
                if j > 1:
                    sgs[j] = emit_ts(j)
                h1 = emit_h1(j - 1, wsgs[j - 1])
                ws[j - 1] = tree_tail(j - 1, h1, nc.gpsimd)
                if j > 1:
                    wsgs[j] = emit_xor(j, sgs[j])
            # chunk 7: all three tree levels split by f-half on DVE so
            # W7[fb0] lands ~0.7us after xor7 and the PE tail starts early
            wv7 = wsgs[7][:].bitcast(BF16).rearrange("p (b f) -> p b f", b=8)
            w7 = wp.tile([P, nfs], BF16, tag="w", name="w7")
            for half in range(2):
                f0, f1 = half * P, (half + 1) * P
                h1h = qp.tile([P, 4, P], BF16, tag="h1h", name=f"h1h{half}")
                nc.vector.tensor_tensor(
                    h1h[:], wv7[:, :4, f0:f1], wv7[:, 4:, f0:f1], OP.add
                )
                h2h = qp.tile([P, 2, P], BF16, tag="h2h", name=f"h2h{half}")
                nc.vector.tensor_tensor(
                    h2h[:], h1h[:, :2], h1h[:, 2:], OP.add
                )
                nc.vector.tensor_tensor(
                    w7[:, f0:f1], h2h[:, 0], h2h[:, 1], OP.add
                )
            ws[7] = w7

            # --- matmul: outT[f, n] = sum_j W_j.T @ xT_j (+bias at evac) ---
            # j outermost: each W chunk feeds the PE as soon as it is
            # decoded, all nfb*nch PSUM banks accumulate concurrently.
            # The last chunk is issued bank-by-bank so evacuation and
            # output DMA overlap the remaining j=7 matmuls.
            for j in range(7):
                for fb in range(nfb):
                    for ch in range(nch):
                        nc.tensor.matmul(
                            pss[(fb, ch)][:],
                            ws[j][:, fb * P : (fb + 1) * P],
                            xts[j][:, ch * MM_N : (ch + 1) * MM_N],
                            start=(j == 0 and (fb, ch) != FILLB),
                            stop=False,
                        )
            # evacuation alternates DVE/ACT (GPSIMD cannot read PSUM) and
            # folds the bias in for free: DVE as tensor_scalar add with a
            # per-partition fp32 bias column, ACT as Identity with the AP
            # bias operand.  Banks pair into double-wide tiles; each pair
            # ships as ONE DMA, fb0 pairs on the gpsimd SWDGE queue and
            # fb1 pairs on SP/HWDGE so the issue pipes overlap.
            k = 0
            for fb in range(nfb):
                bcol = biasc[:, fb : fb + 1]
                for cp2 in range(nch // 2):
                    obw = op_.tile([P, 2 * MM_N], BF16, tag="obw",
                                   name=f"obw{fb}_{cp2}")
                    for half in range(2):
                        ch = 2 * cp2 + half
                        nc.tensor.matmul(
                            pss[(fb, ch)][:],
                            ws[7][:, fb * P : (fb + 1) * P],
                            xts[7][:, ch * MM_N : (ch + 1) * MM_N],
                            start=False,
                            stop=True,
                        )
                        dst = obw[:, half * MM_N : (half + 1) * MM_N]
                        if k % 2 == 0:
                            nc.vector.tensor_scalar(
                                dst, pss[(fb, ch)][:], bcol, None, OP.add
                            )
                        else:
                            nc.scalar.activation(
                                dst, pss[(fb, ch)][:], ACT_F.Identity,
                                bias=bcol,
                            )
                        k += 1
                    deng = nc.gpsimd if fb == 0 else nc.sync
                    deng.dma_start(
                        out_d[fb][:, 2 * cp2 * MM_N : (2 * cp2 + 2) * MM_N],
                        obw[:],
                    )
    nc.compile()
    return nc


_I_PERM = 8 * (np.arange(NX) % G) + np.arange(NX) // G  # i' -> i


def host_prep(x, binary, scale, bias):
    """Layout-only sharding (plus x's fp16 compute-precision cast).
    Returns in_maps for cores 0..7 (pt = c//PF, pf = c%PF)."""
    x2 = np.ascontiguousarray(x.reshape(N_TOK, NX).T)[_I_PERM]  # (NX, N)
    x2 = x2.astype(BF16NP)  # compute dtype
    binary16 = binary.astype(np.int16)  # lossless: codes are 0..255
    in_maps = []
    for c in range(8):
        pt, pf = c // PF, c % PF
        f0 = pf * NFS
        xs = np.ascontiguousarray(x2[:, pt * TOK : (pt + 1) * TOK]).reshape(
            8, P, TOK
        )
        cs = np.ascontiguousarray(
            binary16[:, :, f0 : f0 + NFS].transpose(1, 0, 2)
        ).reshape(P, 8 * NFS)
        ss = np.ascontiguousarray(
            np.broadcast_to(
                (-scale[:, f0 : f0 + NFS].astype(BF16NP)).reshape(1, 8 * NFS),
                (P, 8 * NFS),
            )
        )
        bs = np.ascontiguousarray(
            bias[f0 : f0 + NFS].astype(np.float32).reshape(NFS // P, P).T
        )
        in_maps.append({"xt": xs, "codes": cs, "scales": ss, "biasc": bs})
    return in_maps


def host_assemble(results):
    """results[c]["out"]: (NFB, 128, TOK) -> full (B, T, NF)."""
    outT = np.empty((NF, N_TOK), dtype=np.float32)
    for c in range(8):
        pt, pf = c // PF, c % PF
        o = np.asarray(results[c]["out"], dtype=np.float32).reshape(NFS, TOK)
        outT[pf * NFS : (pf + 1) * NFS, pt * TOK : (pt + 1) * TOK] = o
    return np.ascontiguousarray(outT.T).reshape(B, T, NF)


_NC_CACHE = {}


def _get_nc():
    if "nc" not in _NC_CACHE:
        nc = bacc.Bacc(None, target_bir_lowering=False)
        build_graph(nc)
        _NC_CACHE["nc"] = nc
    return _NC_CACHE["nc"]


def kernel(**inputs):
    from concourse.bass_utils import run_bass_kernel_spmd

    inputs = {k: np.asarray(v) for k, v in inputs.items()}
    in_maps = host_prep(
        inputs["x"], inputs["binary"], inputs["scale"], inputs["bias"]
    )
    res = run_bass_kernel_spmd(_get_nc(), in_maps, core_ids=list(range(8)))
    return host_assemble(res.results)


# revision 16
# speedup vs baseline: 1.1936x; 1.0943x over previous
"""BQuantConv1d Trainium2 kernel.

Math: the reference's per-token LUT + gather is algebraically a matmul:
  out[n, f] = sum_i x[n, i] * W[i, f] + bias[f]
  W[8g+j, f] = sum_b scale[b, f] * (2*bit_{7-j}(binary[b, g, f]) - 1)

Sharding: 2 token-groups x 4 f-groups over 8 cores, no collectives
(host slices inputs / concatenates outputs; layout-only host work).
Contraction order is permuted to i' = j*128 + g (host permutes xT rows to
match) so each decoded weight chunk j lands on contiguous partitions.

Per core:
  - decode W'(1024, 256) from int16 codes with a sign-bit trick:
    W element = +-scale[b, f] exactly, built by XORing the fp16 scale's
    sign bit (scales arrive sign-pre-flipped) with the masked quant bit
    (c << (8+j)) & 0x8000, as int32 SWAR on DVE (bitvec ops are DVE-only
    and 32-bit-only on walrus); the 8-way b-reduction is an fp16 add
    tree with the first level (h1) on DVE and the h2/w levels offloaded
    to the otherwise-idle GPSIMD engine.  Chunks 0/1 run TS/xor in
    b-halves gated on half-sized cd/sc DMAs; chunk 7's whole tree runs
    on DVE split by f-half at every level so the fb=0 matmuls start
    while fb=1 is still reducing.
  - outT[f_shard, n_shard] = W'.T @ xT on the PE in fp16, accumulating
    the 8 contraction chunks across 8 concurrent PSUM banks (f32).
  - The PE p-state ramp (cost model: ~7us of continuous execution
    before full clock) is paid with low-priority zero-matmul fillers
    that keep the PE busy from ~1us instead of bias-seed matmuls; the
    filler bank's real accumulation chain is dep-deferred behind them
    and doubles as tail work.  The bias rides the PSUM evacuation for
    free (DVE tensor_scalar add with a per-partition fp32 bias column /
    ACT Identity activation with an AP bias operand).
  - PSUM pairs evacuated as fp16 (copies alternating DVE/ACT) into
    double-wide tiles; each ch-pair ships as ONE partition-major DMA,
    fb0 pairs on the gpsimd SWDGE queue, fb1 pairs on SP/HWDGE, so the
    two issue pipes overlap in the tail.
"""

import numpy as np

try:
    import concourse.bass as bass  # noqa: F401
except ImportError:
    import sys

    sys.path.insert(0, "/opt/trn_rl_repo")
    import concourse.bass as bass  # noqa: F401

import concourse.bacc as bacc
import concourse.mybir as mybir
import concourse.tile as tile

B, T, NX, NF = 2, 2048, 1024, 1024
N_TOK = B * T
BITS = 8
G = NX // 8  # 128 code groups
PT, PF = 2, 4  # token-parallel x feature-parallel
TOK = N_TOK // PT  # tokens per core
NFS = NF // PF  # output features per core
P = 128
MM_N = 512  # moving free dim per matmul
N_FILL = 24  # PE warm-up/pacing zero-matmuls (tuned against TimelineSim)

AX = mybir.AxisListType
OP = mybir.AluOpType
F32 = mybir.dt.float32
BF16 = mybir.dt.float16  # compute dtype (fp16: same SWAR, more mantissa)
I16 = mybir.dt.int16
I32 = mybir.dt.int32
ACT_F = mybir.ActivationFunctionType
BF16NP = np.float16

MSK = -2147450880  # 0x80008000 as int32


def build_graph(nc, tok=TOK, nfs=NFS):
    nfb = nfs // P  # f blocks of 128 (2)
    nch = tok // MM_N  # moving chunks (4)
    xt_d = nc.dram_tensor("xt", (8, P, tok), BF16, kind="ExternalInput")
    cd_d = nc.dram_tensor("codes", (P, 8 * nfs), I16, kind="ExternalInput")
    sc_d = nc.dram_tensor("scales", (P, 8 * nfs), BF16, kind="ExternalInput")
    bi_d = nc.dram_tensor("biasc", (P, nfb), F32, kind="ExternalInput")
    out_d = nc.dram_tensor("out", (nfb, P, tok), BF16, kind="ExternalOutput")
    # fb0 banks host the warm-up fillers; their real chains get dep-deferred
    # behind their fillers and become gap-fill work mid-stream
    FILL_BANKS = [(0, ch) for ch in range(nch)]

    with tile.TileContext(nc) as tc:
        with (
            tc.tile_pool(name="xp", bufs=8) as xp,
            tc.tile_pool(name="cp", bufs=8) as cp,
            tc.tile_pool(name="wp", bufs=8) as wp,
            tc.tile_pool(name="qp", bufs=6) as qp,
            tc.tile_pool(name="cst", bufs=1) as cst,
            tc.tile_pool(name="op", bufs=8) as op_,
            tc.tile_pool(name="pp", bufs=8, space="PSUM") as pp,
        ):
            # --- loads; codes/scales first, halves interleaved: chunk 0/1's
            # decode runs in b-halves gated on each half-DMA ---
            H = 4 * nfs
            cd = cp.tile([P, 8 * nfs], I16, tag="cd")
            sc_bc = cst.tile([P, 8 * nfs], BF16, tag="sc_bc")
            nc.sync.dma_start(cd[:, :H], cd_d[:, :H])
            nc.sync.dma_start(sc_bc[:, :H], sc_d[:, :H])
            nc.sync.dma_start(cd[:, H:], cd_d[:, H:])
            nc.sync.dma_start(sc_bc[:, H:], sc_d[:, H:])
            biasc = cst.tile([P, nfb], F32, tag="biasc")
            nc.sync.dma_start(biasc[:], bi_d[:])
            zero_m = cst.tile([P, MM_N], BF16, tag="zero_m")
            nc.vector.memset(zero_m[:], 0.0)  # DVE: idle until cd lands
            xts = []
            for j in range(8):
                xt = xp.tile([P, tok], BF16, tag="xt")
                nc.sync.dma_start(xt[:], xt_d[j])
                xts.append(xt)

            # PSUM banks.  FILLB is reset by the first zero-filler; every
            # other bank starts accumulation at its j=0 matmul.
            pss = {}
            for fb in range(nfb):
                for ch in range(nch):
                    ps = pp.tile([P, MM_N], F32, tag="ps", name=f"ps{fb}_{ch}")
                    pss[(fb, ch)] = ps

            # --- PE warm-up fillers: zero-weight matmuls spread across the
            # fb0 banks.  Low priority: the scheduler runs them only when
            # no real matmul is ready; they pay the cost model's ~10us PE
            # p-state ramp starting at ~0.9us, while the PE would
            # otherwise idle waiting for the first decoded W chunk.
            with tc.high_priority(offset=-(10**6)):
                for i in range(N_FILL):
                    bank = FILL_BANKS[i % len(FILL_BANKS)]
                    nc.tensor.matmul(
                        pss[bank][:], zero_m[:, :P], zero_m[:],
                        start=(i < len(FILL_BANKS)), stop=False,
                    )

            # --- decode W chunks ---
            # Sign-bit trick: masked quant bit (inverted) XORed onto the
            # fp16 scale's sign gives +-scale exactly.  Bitvec ops are
            # DVE-only and 32-bit-only on walrus, so they run as int32 SWAR
            # over int16-lane pairs: a left shift by 8+j sources each
            # lane's bit 15 from within the same lane, and the 0x80008000
            # mask keeps only the two sign bits.  The bit inversion is
            # folded into a one-time sign-flip of the scale tile:
            #   ((c << (8+j)) & M) ^ (sc ^ M)  ==  ((~c << (8+j)) & M) ^ sc
            def emit_ts(j):
                sg = qp.tile([P, 8 * nfs], I16, tag="sg", name=f"sg{j}")
                nc.vector.tensor_scalar(
                    sg[:].bitcast(I32), cd[:].bitcast(I32), 8 + j, MSK,
                    OP.logical_shift_left, OP.bitwise_and,
                )
                return sg

            def emit_xor(j, sg):
                wsg = qp.tile([P, 8 * nfs], I16, tag="wsg", name=f"wsg{j}")
                nc.vector.tensor_tensor(
                    wsg[:].bitcast(I32), sg[:].bitcast(I32),
                    sc_bc[:].bitcast(I32), OP.bitwise_xor,
                )
                return wsg

            def emit_h1(j, wsg):
                wv = wsg[:].bitcast(BF16)
                h1 = qp.tile([P, 4 * nfs], BF16, tag="h1", name=f"h1_{j}")
                nc.vector.tensor_tensor(
                    h1[:], wv[:, : 4 * nfs], wv[:, 4 * nfs :], OP.add
                )
                return h1

            def tree_tail(j, h1, teng):
                h2 = qp.tile([P, 2 * nfs], BF16, tag="h2", name=f"h2_{j}")
                teng.tensor_tensor(
                    h2[:], h1[:, : 2 * nfs], h1[:, 2 * nfs :], OP.add
                )
                w = wp.tile([P, nfs], BF16, tag="w", name=f"w{j}")
                teng.tensor_tensor(w[:], h2[:, :nfs], h2[:, nfs:], OP.add)
                return w

            # b-reduction fp16 add tree: h1 on DVE; h2/w on GPSIMD for
            # chunks 0-6.  The next chunk's TS is emitted BETWEEN xor_j and
            # h1_j: it is always ready, so it fills the write-ack window
            # after xor_j and the scheduler then runs h1_j instead of
            # parking the next 1.1us xor in front of it.
            ws = {}
            # chunks 0/1 in b-halves so each half starts on its half-DMA
            # (fills the DVE while the second cd/sc halves are in flight)
            sgs, wsgs = {}, {}
            for j in (0, 1):
                sg = qp.tile([P, 8 * nfs], I16, tag="sg", name=f"sg{j}")
                wsg = qp.tile([P, 8 * nfs], I16, tag="wsg", name=f"wsg{j}")
                for half in range(2):
                    lo, hi = half * H, (half + 1) * H
                    nc.vector.tensor_scalar(
                        sg[:, lo:hi].bitcast(I32), cd[:, lo:hi].bitcast(I32),
                        8 + j, MSK, OP.logical_shift_left, OP.bitwise_and,
                    )
                    nc.vector.tensor_tensor(
                        wsg[:, lo:hi].bitcast(I32),
                        sg[:, lo:hi].bitcast(I32),
                        sc_bc[:, lo:hi].bitcast(I32), OP.bitwise_xor,
                    )
                sgs[j], wsgs[j] = sg, wsg
            for j in range(1, 8):
                if j > 1:
                    sgs[j] = emit_ts(j)
                h1 = emit_h1(j - 1, wsgs[j - 1])
                ws[j - 1] = tree_tail(j - 1, h1, nc.gpsimd)
                if j > 1:
                    wsgs[j] = emit_xor(j, sgs[j])
            # chunk 7: all three tree levels split by f-half on DVE so
            # W7[fb0] lands ~0.7us after xor7 and the PE tail starts early
            wv7 = wsgs[7][:].bitcast(BF16).rearrange("p (b f) -> p b f", b=8)
            w7 = wp.tile([P, nfs], BF16, tag="w", name="w7")
            for half in range(2):
                f0, f1 = half * P, (half + 1) * P
                h1h = qp.tile([P, 4, P], BF16, tag="h1h", name=f"h1h{half}")
                nc.vector.tensor_tensor(
                    h1h[:], wv7[:, :4, f0:f1], wv7[:, 4:, f0:f1], OP.add
                )
                h2h = qp.tile([P, 2, P], BF16, tag="h2h", name=f"h2h{half}")
                nc.vector.tensor_tensor(
                    h2h[:], h1h[:, :2], h1h[:, 2:], OP.add
                )
                nc.vector.tensor_tensor(
                    w7[:, f0:f1], h2h[:, 0], h2h[:, 1], OP.add
                )
            ws[7] = w7

            # --- matmul: outT[f, n] = sum_j W_j.T @ xT_j (+bias at evac) ---
            # j outermost: each W chunk feeds the PE as soon as it is
            # decoded, all nfb*nch PSUM banks accumulate concurrently.
            # The last chunk is issued bank-by-bank so evacuation and
            # output DMA overlap the remaining j=7 matmuls.
            for j in range(7):
                for fb in range(nfb):
                    for ch in range(nch):
                        nc.tensor.matmul(
                            pss[(fb, ch)][:],
                            ws[j][:, fb * P : (fb + 1) * P],
                            xts[j][:, ch * MM_N : (ch + 1) * MM_N],
                            start=(j == 0 and (fb, ch) not in FILL_BANKS),
                            stop=False,
                        )
            # evacuation alternates DVE/ACT (GPSIMD cannot read PSUM) and
            # folds the bias in for free: DVE as tensor_scalar add with a
            # per-partition fp32 bias column, ACT as Identity with the AP
            # bias operand.  Banks pair into double-wide tiles; each pair
            # ships as ONE DMA, fb0 pairs on the gpsimd SWDGE queue and
            # fb1 pairs on SP/HWDGE so the issue pipes overlap.
            k = 0
            for fb in range(nfb):
                bcol = biasc[:, fb : fb + 1]
                for cp2 in range(nch // 2):
                    obw = op_.tile([P, 2 * MM_N], BF16, tag="obw",
                                   name=f"obw{fb}_{cp2}")
                    for half in range(2):
                        ch = 2 * cp2 + half
                        nc.tensor.matmul(
                            pss[(fb, ch)][:],
                            ws[7][:, fb * P : (fb + 1) * P],
                            xts[7][:, ch * MM_N : (ch + 1) * MM_N],
                            start=False,
                            stop=True,
                        )
                        dst = obw[:, half * MM_N : (half + 1) * MM_N]
                        if k % 2 == 0:
                            nc.vector.tensor_scalar(
                                dst, pss[(fb, ch)][:], bcol, None, OP.add
                            )
                        else:
                            nc.scalar.activation(
                                dst, pss[(fb, ch)][:], ACT_F.Identity,
                                bias=bcol,
                            )
                        k += 1
                    deng = nc.gpsimd if fb == 0 else nc.sync
                    deng.dma_start(
                        out_d[fb][:, 2 * cp2 * MM_N : (2 * cp2 + 2) * MM_N],
                        obw[:],
                    )
    nc.compile()
    return nc


_I_PERM = 8 * (np.arange(NX) % G) + np.arange(NX) // G  # i' -> i


def host_prep(x, binary, scale, bias):
    """Layout-only sharding (plus x's fp16 compute-precision cast).
    Returns in_maps for cores 0..7 (pt = c//PF, pf = c%PF)."""
    x2 = np.ascontiguousarray(x.reshape(N_TOK, NX).T)[_I_PERM]  # (NX, N)
    x2 = x2.astype(BF16NP)  # compute dtype
    binary16 = binary.astype(np.int16)  # lossless: codes are 0..255
    in_maps = []
    for c in range(8):
        pt, pf = c // PF, c % PF
        f0 = pf * NFS
        xs = np.ascontiguousarray(x2[:, pt * TOK : (pt + 1) * TOK]).reshape(
            8, P, TOK
        )
        cs = np.ascontiguousarray(
            binary16[:, :, f0 : f0 + NFS].transpose(1, 0, 2)
        ).reshape(P, 8 * NFS)
        ss = np.ascontiguousarray(
            np.broadcast_to(
                (-scale[:, f0 : f0 + NFS].astype(BF16NP)).reshape(1, 8 * NFS),
                (P, 8 * NFS),
            )
        )
        bs = np.ascontiguousarray(
            bias[f0 : f0 + NFS].astype(np.float32).reshape(NFS // P, P).T
        )
        in_maps.append({"xt": xs, "codes": cs, "scales": ss, "biasc": bs})
    return in_maps


def host_assemble(results):
    """results[c]["out"]: (NFB, 128, TOK) -> full (B, T, NF)."""
    outT = np.empty((NF, N_TOK), dtype=np.float32)
    for c in range(8):
        pt, pf = c // PF, c % PF
        o = np.asarray(results[c]["out"], dtype=np.float32).reshape(NFS, TOK)
        outT[pf * NFS : (pf + 1) * NFS, pt * TOK : (pt + 1) * TOK] = o
    return np.ascontiguousarray(outT.T).reshape(B, T, NF)


_NC_CACHE = {}


def _get_nc():
    if "nc" not in _NC_CACHE:
        nc = bacc.Bacc(None, target_bir_lowering=False)
        build_graph(nc)
        _NC_CACHE["nc"] = nc
    return _NC_CACHE["nc"]


def kernel(**inputs):
    from concourse.bass_utils import run_bass_kernel_spmd

    inputs = {k: np.asarray(v) for k, v in inputs.items()}
    in_maps = host_prep(
        inputs["x"], inputs["binary"], inputs["scale"], inputs["bias"]
    )
    res = run_bass_kernel_spmd(_get_nc(), in_maps, core_ids=list(range(8)))
    return host_assemble(res.results)
